# revision 21
# baseline (speedup 1.0000x reference)
"""Trainium2 Bass kernel for nn_Attention_48610439856262.

Gated attention block:
    qkv = x @ W_qkv ; gate = x @ W_gate ; s = e @ W_s (added to k)
    attn = softmax(q @ (k+s).T * D**-0.5) ; out = (attn @ v) * gate
    y = out @ W_proj + b_proj

Sharding (8 cores, tensor-parallel over heads):
  Core c owns heads {2c, 2c+1} = feature columns 128c:128c+128 of the
  (H, D)-structured feature axis.  Each core computes q/k+s/v/gate for its
  128 feature columns over all 4096 tokens, runs attention for its 2 heads,
  multiplies by its gate slice, and computes a PARTIAL projection
  y_c = gated_c @ W_proj[128c:128c+128, :]  ->  [4096, 1024].
  The host sums the 8 partials and adds b_proj (no device collectives).

All matmuls are float32r: separate-LDWEIGHTS dtypes (bf16/fp8) measured
+160 ns/MM on HW because every matmul here rotates its stationary operand
and the per-matmul LDWEIGHTS does not overlap (fp32r self-loads; the
no-exec cost model charges LDWEIGHTS zero, so only HW shows this).
PSUM accumulation fp32.  y goes out as bf16 (host reduces in fp32).

Device layout:
  xT, eT   [1024, 4096] fp32 wire (host pre-transposes)
  qT/kpsT/gT/gatedT  SBUF [128 feat, 4096 tok] f32r; k+s is fused into
           one PSUM accumulation group (16 matmuls), no DVE add.
  v        feature-major matmul + PE transpose; stored per 128-token
           block as [v_h0(64) | ones | v_h1(64) | ones] so the
           attn @ v_aug PSUM row 64 holds the softmax denominators,
           partition-aligned with the gate slice for h0 (h1 needs one
           SBUF->SBUF DMA partition shift).
  scores   PSUM [128 keys, 1024] fp32 = [h0 512q | h1 512q]: the two
           heads' score matmuls are row-tiled (contraction 64 ->
           tile_position (0,0)/(64,0)) and adjacent in issue order so
           they overlap on the PE array; one 1024-wide ACT Exp call
           (fused *SCALE; no max-subtraction: scores ~N(0,0.6), |s|<6,
           exp safe in fp32) covers both heads.

Emission order software-pipelines everything: per batch, attention on
the first 8 key-blocks of (nh0, jj0) is emitted right after the first
two A-chunks so ACT starts ~25us earlier; scores(mb+1) is emitted before
PV(mb) so the in-order PE queue never head-of-line blocks on ACT(mb);
batch b1's phase A interleaves under batch b0's attention; projection
reuses the pv PSUM banks after each (b, nh) drain.
PSUM budget: scores 2x[128,1024] (4 banks) + pv0/pv1 [128,512] (2) +
acc (1) + trp (1) = 8.
"""

import numpy as np
import ml_dtypes

BF16 = ml_dtypes.bfloat16

B, N, C, H, D = 2, 2048, 1024, 16, 64
T = B * N              # 4096 tokens
NCORES = 8
F = 128                # feature columns per core (2 heads x 64)
SCALE = D ** -0.5
KC = C // 128          # 8 contraction chunks
TC = T // 512          # 8 token chunks of 512
MB = N // 128          # 16 key blocks per sequence
TB = T // 128          # 32 token blocks

_cache: dict = {}


def _build_program(reps=1):
    import concourse.bacc as bacc
    import concourse.tile as tile
    from concourse import mybir
    from concourse.masks import make_identity

    f32 = mybir.dt.float32
    f32r = mybir.dt.float32r
    obf16 = mybir.dt.bfloat16

    nc = bacc.Bacc("TRN2", target_bir_lowering=False, debug=False,
                   num_devices=NCORES)

    xT = nc.dram_tensor("xT", [C, T], f32r, kind="ExternalInput").ap()
    eT = nc.dram_tensor("eT", [C, T], f32r, kind="ExternalInput").ap()
    wq = nc.dram_tensor("wq", [C, F], f32r, kind="ExternalInput").ap()
    wk = nc.dram_tensor("wk", [C, F], f32r, kind="ExternalInput").ap()
    wv = nc.dram_tensor("wv", [C, F], f32r, kind="ExternalInput").ap()
    ws = nc.dram_tensor("ws", [C, F], f32r, kind="ExternalInput").ap()
    wg = nc.dram_tensor("wg", [C, F], f32r, kind="ExternalInput").ap()
    wp = nc.dram_tensor("wp", [F, C], f32r, kind="ExternalInput").ap()
    y = nc.dram_tensor("y", [T, C], obf16, kind="ExternalOutput").ap()

    Exp = mybir.ActivationFunctionType.Exp

    with tile.TileContext(nc) as tc:
        with tc.tile_pool(name="persist", bufs=1) as persist, \
             tc.tile_pool(name="psum", bufs=1, space="PSUM") as psum, \
             tc.tile_pool(name="xa", bufs=10) as xa_pool, \
             tc.tile_pool(name="ea", bufs=10) as ea_pool, \
             tc.tile_pool(name="vt", bufs=2) as vt_pool, \
             tc.tile_pool(name="pt", bufs=6) as pt_pool, \
             tc.tile_pool(name="small", bufs=2) as small, \
             tc.tile_pool(name="yout", bufs=4) as y_pool:
            # Weights, contraction-chunked: [128 k-part, KC, 128 cols]
            w_sb = {}
            for name, src in (("wq", wq), ("wk", wk), ("wv", wv),
                              ("ws", ws), ("wg", wg)):
                t_ = persist.tile([128, KC, F], f32r, tag=name,
                                  name=f"w_{name}")
                nc.sync.dma_start(out=t_,
                                  in_=src.rearrange("(k p) f -> p k f", p=128))
                w_sb[name] = t_
            ident = persist.tile([128, 128], f32, tag="ident")
            make_identity(nc, ident)

            wp_sb = persist.tile([F, C], f32r, tag="wp")
            nc.sync.dma_start(out=wp_sb, in_=wp)

            qT_s = persist.tile([128, T], f32r, tag="qT")
            kpsT_s = persist.tile([128, T], f32r, tag="kpsT")
            gT_s = persist.tile([128, T], obf16, tag="gT")
            gatedT_s = persist.tile([128, T], f32r, tag="gatedT")
            # v_aug per 128-token block: [v_h0 | 1 | v_h1 | 1] -> the
            # attn@v_aug PSUM row 64 is the softmax denominator.
            v_s = persist.tile([128, TB, 130], f32r, tag="v")
            ones_col = persist.tile([128, TB], f32, tag="ones_col")
            nc.vector.memset(ones_col, 1.0)
            nc.vector.tensor_copy(v_s[:, :, 64], ones_col)
            nc.vector.tensor_copy(v_s[:, :, 129], ones_col)

            def emit_a_chunk(t):
                """Phase A for one 512-token chunk."""
                sl = slice(t * 512, (t + 1) * 512)
                xt = [xa_pool.tile([128, 512], f32r, tag="xt",
                                   name=f"xt{t}_{k}") for k in range(KC)]
                for k in range(KC):
                    nc.sync.dma_start(out=xt[k],
                                      in_=xT[k * 128:(k + 1) * 128, sl])
                et = [ea_pool.tile([128, 512], f32r, tag="et",
                                   name=f"et{t}_{k}") for k in range(KC)]
                for k in range(KC):
                    nc.sync.dma_start(out=et[k],
                                      in_=eT[k * 128:(k + 1) * 128, sl])
                for out_name, dst in (("q", qT_s), ("g", gT_s)):
                    acc = psum.tile([128, 512], f32, tag="acc",
                                    name=f"acc_{out_name}")
                    w_t = w_sb["w" + out_name]
                    for k in range(KC):
                        nc.tensor.matmul(acc, w_t[:, k, :], xt[k],
                                         start=(k == 0), stop=(k == KC - 1))
                    nc.vector.tensor_copy(dst[:, sl], acc)
                # k+s fused in one PSUM accumulation group
                acc = psum.tile([128, 512], f32, tag="acc", name="acc_kps")
                for k in range(KC):
                    nc.tensor.matmul(acc, w_sb["wk"][:, k, :], xt[k],
                                     start=(k == 0), stop=False)
                for k in range(KC):
                    nc.tensor.matmul(acc, w_sb["ws"][:, k, :], et[k],
                                     start=False, stop=(k == KC - 1))
                nc.vector.tensor_copy(kpsT_s[:, sl], acc)
                # v feature-major, then transpose to token-major v_aug
                acc = psum.tile([128, 512], f32, tag="acc", name="acc_v")
                for k in range(KC):
                    nc.tensor.matmul(acc, w_sb["wv"][:, k, :], xt[k],
                                     start=(k == 0), stop=(k == KC - 1))
                vt_tmp = vt_pool.tile([128, 512], f32, tag="vt")
                nc.vector.tensor_copy(vt_tmp, acc)
                for j in range(4):
                    tb = t * 4 + j
                    pt_ = psum.tile([128, 128], f32, tag="trp", name="tr")
                    nc.tensor.transpose(pt_, vt_tmp[:, j * 128:(j + 1) * 128],
                                        ident)
                    nc.vector.tensor_copy(v_s[:, tb, 0:64], pt_[:, 0:64])
                    nc.vector.tensor_copy(v_s[:, tb, 65:129], pt_[:, 64:128])

            class Section:
                """Attention for one (b, nh, jj): 512 queries x all keys.

                Emission is software-pipelined: feed() emits scores+exp for
                the given key-blocks, holding PV(mb) back until
                scores(mb+1) is emitted; finish() drains and gates.
                """

                def __init__(self, b, nh, jj):
                    self.b, self.nh, self.jj = b, nh, jj
                    self.nsl = slice(b * N + nh * 1024 + jj * 512,
                                     b * N + nh * 1024 + (jj + 1) * 512)
                    self.psv = [psum.tile([128, 512], f32, tag=f"pv{h}",
                                          name=f"pv{h}_{b}{nh}{jj}")
                                for h in range(2)]
                    self.pts = {}
                    self.pending = []

                def _scores(self, mb):
                    b = self.b
                    msl = slice(b * N + mb * 128, b * N + mb * 128 + 128)
                    ps_s = psum.tile([128, 1024], f32, tag="scores",
                                     name="scores", bufs=2)
                    # two heads row-tiled (rows 0-63 / 64-127), adjacent
                    for h in range(2):
                        hsl = slice(h * 64, (h + 1) * 64)
                        nc.tensor.matmul(ps_s[:, h * 512:(h + 1) * 512],
                                         kpsT_s[hsl, msl],
                                         qT_s[hsl, self.nsl],
                                         start=True, stop=True)
                    pt = pt_pool.tile([128, 1024], f32r, tag="pT")
                    nc.scalar.activation(pt, ps_s, Exp, scale=SCALE)
                    self.pts[mb] = pt

                def _pv(self, mb):
                    pt = self.pts.pop(mb)
                    for h in range(2):
                        nc.tensor.matmul(
                            self.psv[h][0:65, :],
                            v_s[:, self.b * MB + mb, h * 65:h * 65 + 65],
                            pt[:, h * 512:(h + 1) * 512],
                            start=(mb == 0), stop=(mb == MB - 1))

                def feed(self, mbs):
                    for mb in mbs:
                        self._scores(mb)
                        self.pending.append(mb)
                        if len(self.pending) > 1:
                            self._pv(self.pending.pop(0))

                def finish(self):
                    while self.pending:
                        self._pv(self.pending.pop(0))
                    for h in range(2):
                        hsl = slice(h * 64, (h + 1) * 64)
                        rs = small.tile([1, 512], f32, tag="rs")
                        nc.vector.reciprocal(rs, self.psv[h][64:65, :])
                        rb = small.tile([64, 512], f32, tag="rb")
                        nc.gpsimd.partition_broadcast(rb, rs)
                        tmp = small.tile([64, 512], f32, tag="tmp")
                        nc.vector.tensor_mul(tmp, self.psv[h][0:64, :], rb)
                        if h == 0:
                            # partitions already aligned with gate rows
                            nc.vector.tensor_mul(gatedT_s[hsl, self.nsl],
                                                 tmp, gT_s[hsl, self.nsl])
                        else:
                            pvn = small.tile([128, 512], f32, tag="pvn")
                            nc.sync.dma_start(out=pvn[hsl, :], in_=tmp)
                            nc.vector.tensor_mul(gatedT_s[hsl, self.nsl],
                                                 pvn[hsl, :],
                                                 gT_s[hsl, self.nsl])

            def emit_proj_pair(tb):
                """Projection for one 128-token block -> full [128, 1024]
                output row.  Rides the scores-tag PSUM rotation so it can
                interleave with the next attention section's mb loop
                without touching the pv accumulator banks."""
                py_ = psum.tile([128, 1024], f32, tag="scores", bufs=2,
                                name="proj")
                for j in range(2):
                    nc.tensor.matmul(
                        py_[:, j * 512:(j + 1) * 512],
                        gatedT_s[:, tb * 128:(tb + 1) * 128],
                        wp_sb[:, j * 512:(j + 1) * 512],
                        start=True, stop=True)
                yt = y_pool.tile([128, 1024], obf16, tag="yt")
                nc.vector.tensor_copy(yt, py_)
                nc.sync.dma_start(out=y[tb * 128:(tb + 1) * 128, :], in_=yt)

            def feed_interleaved(section, mbs, proj_tbs):
                """Feed a section's key-blocks with projection pairs of the
                PREVIOUS (b, nh) woven between them (same scores-tag
                rotation, so ACT never starves while proj drains)."""
                proj_tbs = list(proj_tbs)
                for mb in mbs:
                    section.feed([mb])
                    if proj_tbs:
                        emit_proj_pair(proj_tbs.pop(0))
                for tb in proj_tbs:
                    emit_proj_pair(tb)

            pending_proj = []   # tb blocks whose projection is owed
            for _rep in range(reps):
                for b in range(B):
                    t0 = b * (TC // B)
                    # A-chunks t0, t0+1 cover queries of (nh0, jj0) and
                    # key-blocks 0..7; start attention right after them.
                    emit_a_chunk(t0)
                    emit_a_chunk(t0 + 1)
                    s00 = Section(b, 0, 0)
                    feed_interleaved(s00, range(0, 8), pending_proj)
                    pending_proj = []
                    emit_a_chunk(t0 + 2)
                    s00.feed(range(8, 12))
                    emit_a_chunk(t0 + 3)
                    s00.feed(range(12, 16))
                    s00.finish()
                    s01 = Section(b, 0, 1)
                    s01.feed(range(MB))
                    s01.finish()
                    s10 = Section(b, 1, 0)
                    feed_interleaved(s10, range(MB),
                                     range(b * 16, b * 16 + 8))
                    s10.finish()
                    s11 = Section(b, 1, 1)
                    s11.feed(range(MB))
                    s11.finish()
                    pending_proj = list(range(b * 16 + 8, b * 16 + 16))
            # tail: the final (b, nh) projection has no following section
            for tb in pending_proj:
                emit_proj_pair(tb)

    nc.compile()
    return nc


def _get_nc():
    if "nc" not in _cache:
        _cache["nc"] = _build_program()
    return _cache["nc"]


def _get_exec():
    """Compile once; cache a persistent sharded executable.

    Mirrors concourse.bass2jax.run_bass_via_pjrt's multi-core path, but
    keeps the jitted callable (and device-resident zero output buffers)
    alive so repeat kernel() calls skip XLA/walrus recompilation.  No
    donation: the kernel writes every element of y, so the zero buffers
    are never read and can be reused across calls.
    """
    if "exec" in _cache:
        return _cache["exec"]
    import jax
    from jax.experimental.shard_map import shard_map
    from jax.sharding import Mesh, PartitionSpec
    from concourse import mybir
    from concourse.bass2jax import (_bass_exec_p, install_neuronx_cc_hook,
                                    partition_id_tensor)

    nc = _get_nc()
    install_neuronx_cc_hook()
    partition_name = (nc.partition_id_tensor.name
                      if nc.partition_id_tensor else None)
    in_names, out_names, out_avals = [], [], []
    for alloc in nc.m.functions[0].allocations:
        if not isinstance(alloc, mybir.MemoryLocationSet):
            continue
        name = alloc.memorylocations[0].name
        if alloc.kind == "ExternalInput":
            if name != partition_name:
                in_names.append(name)
        elif alloc.kind == "ExternalOutput":
            out_names.append(name)
            out_avals.append(jax.core.ShapedArray(
                tuple(alloc.tensor_shape), mybir.dt.np(alloc.dtype)))
    n_params, n_outs = len(in_names), len(out_names)
    bind_in_names = tuple(in_names + out_names +
                          ([partition_name] if partition_name else []))

    def _body(*args):
        operands = list(args)
        if partition_name is not None:
            operands.append(partition_id_tensor())
        outs = _bass_exec_p.bind(
            *operands,
            out_avals=tuple(out_avals),
            in_names=bind_in_names,
            out_names=tuple(out_names),
            lowering_input_output_aliases=(),
            sim_require_finite=True,
            sim_require_nnan=True,
            nc=nc,
        )
        return tuple(outs)

    devices = jax.devices()[:NCORES]
    mesh = Mesh(np.asarray(devices), ("core",))
    in_specs = (PartitionSpec("core"),) * (n_params + n_outs)
    out_specs = (PartitionSpec("core"),) * n_outs
    sharded = jax.jit(shard_map(_body, mesh=mesh, in_specs=in_specs,
                                out_specs=out_specs, check_rep=False),
                      keep_unused=True)
    zeros_dev = [
        jax.device_put(
            np.zeros((NCORES * a.shape[0], *a.shape[1:]), a.dtype),
            jax.sharding.NamedSharding(mesh, PartitionSpec("core")))
        for a in out_avals]
    reduce_fn = jax.jit(
        lambda a: a.reshape(NCORES, T, C).astype(np.float32).sum(axis=0))
    ex = {"fn": sharded, "in_names": in_names, "out_names": out_names,
          "out_avals": out_avals, "mesh": mesh, "zeros_dev": zeros_dev,
          "spec": PartitionSpec("core"), "reduce": reduce_fn}
    _cache["exec"] = ex
    return ex


def _run_on_device(in_maps):
    """Run the cached executable; returns per-core output dicts."""
    ex = _get_exec()
    concat_in = [
        np.concatenate([np.asarray(in_maps[c][name])
                        for c in range(NCORES)], axis=0)
        for name in ex["in_names"]]
    out = ex["fn"](*concat_in, *ex["zeros_dev"])
    return [
        {name: np.asarray(out[i]).reshape(NCORES, *ex["out_avals"][i].shape)[c]
         for i, name in enumerate(ex["out_names"])}
        for c in range(NCORES)]


def _make_in_maps(x, e, W_qkv, W_s, W_gate, W_proj):
    xT = np.ascontiguousarray(x.reshape(T, C).T, dtype=np.float32)
    eT = np.ascontiguousarray(e.reshape(T, C).T, dtype=np.float32)
    in_maps = []
    for c in range(NCORES):
        fs = slice(F * c, F * (c + 1))
        in_maps.append({
            "xT": xT,
            "eT": eT,
            "wq": np.ascontiguousarray(W_qkv[:, fs], dtype=np.float32),
            "wk": np.ascontiguousarray(W_qkv[:, C:][:, fs], dtype=np.float32),
            "wv": np.ascontiguousarray(W_qkv[:, 2 * C:][:, fs],
                                       dtype=np.float32),
            "ws": np.ascontiguousarray(W_s[:, fs], dtype=np.float32),
            "wg": np.ascontiguousarray(W_gate[:, fs], dtype=np.float32),
            "wp": np.ascontiguousarray(W_proj[fs, :], dtype=np.float32),
        })
    return in_maps


def kernel(x, e, W_qkv, W_s, W_gate, W_proj, b_proj):
    ex = _get_exec()
    in_maps = _make_in_maps(np.asarray(x), np.asarray(e), np.asarray(W_qkv),
                            np.asarray(W_s), np.asarray(W_gate),
                            np.asarray(W_proj))
    concat_in = [
        np.concatenate([np.asarray(in_maps[c][name])
                        for c in range(NCORES)], axis=0)
        for name in ex["in_names"]]
    out = ex["fn"](*concat_in, *ex["zeros_dev"])
    iy = ex["out_names"].index("y")
    y_sum = np.asarray(ex["reduce"](out[iy]))   # cross-core partial sum
    y_sum = y_sum + np.asarray(b_proj, dtype=np.float32)
    return y_sum.reshape(B, N, C).astype(np.float32)
